# revision 31
# baseline (speedup 1.0000x reference)
"""Bass/Trainium2 kernel for nn_CoreAttention (NTK causal attention with
linear phi-correction), SPMD over 8 NeuronCores.

Math (per batch b, head h; q,k,v: [n, d]; Z=phi_kv[h]: [d,d]; kk=|phi_k[h]|: [d,1]):
    phi_q  = ELU(q / d**0.25) + 1        ~= relu(q / d**0.25 + 1)  (rel err 1.3e-4)
    S      = q @ k.T / sqrt(d)
    A      = exp(S) * causal             # max-shift invariant -> use m=0
    num    = A @ v + phi_q @ Z
    den    = A @ ones + phi_q @ kk
    ctx    = num / den                   # division done on host

Sharding: batch*head pairs (32) split 4-per-core across 8 cores. No
cross-core communication.

Engine orchestration (the score exp is the serial bottleneck if left on
ScalarE alone, so it is split):
    PE      QK^T scores (S^T/64 in PSUM), V-stationary AV accumulate into
            num^T[65, q] (65-col stationary weights), phi@[Z|kk] opener.
    ScalarE exact exp (scale=8 folded) for 2 of every 3 score groups;
            PSUM->SBUF copies of finished num^T blocks.
    VectorE custom fused exp for the remaining groups:
            pass1 EXP_HORNER4_ANT: p = 1+u(c1+u(c2+u(c3+u*c4))) ~ e^u
            pass2 SQ3_ANT: p^8 = e^{8u} = e^{S/sqrt(d)}  (u = S/64 scores,
            q and k are host-prescaled by 1/8 each; end-to-end rel err
            ~1.2e-3, validated against the reference)
    GpSimd  causal masks on the 128x128 diagonal blocks, phi_q relu.
Host: input transposes / fp16 casts, final num/den division + transpose.
"""

import dataclasses
import math

import numpy as np

import concourse.bacc as bacc
import concourse.mybir as mybir
from concourse.tile import TileContext

SEQ, BS, H, D = 2048, 2, 16, 64
N_CORES = 8
NPAIR = BS * H            # 32 (b,h) pairs
PPC = NPAIR // N_CORES    # 4 pairs per core
P = 128                   # partition tile
NKT = SEQ // P            # 16 k tiles per pair
QB = 512                  # q-block width (one PSUM bank of fp32)
NQB = SEQ // QB           # 4 q blocks
QT_PER_B = QB // P        # 4 q tiles per block
DA = D + 1                # v augmented with ones column

_C = 1.0 / (D ** 0.25)        # phi input scale (on unscaled q)
_PHI_SCALE = 2.0 ** -7        # keep phi*Z product in fp16 normal range
_QK_PRE = 1.0 / 8.0           # host prescale on q and k: u = S/64 in PSUM
_ACT_SCALE = 8.0              # exp(8*u) = exp(S/sqrt(d)) on ScalarE
WARM_MMS = 19                 # HAM warm-up burst length
WARM2_MMS = 10                # post-prefill filler: lets ScalarE work 2-3
                              # steps ahead before the PE enters the loop
DVE_EVERY = 4                 # every 4th score group exps on VectorE
QB_ORDER = [3, 2, 1, 0]       # qb schedule: diag (masked) groups arrive
                              # only after the pipeline has built backlog,
                              # and a num^T bank's copy-out sits far from
                              # its next accumulation

# minimax deg-4 (c0=1) fit of e^u on [-0.75, 0.75], rel err 1.43e-4
_PC1 = 0.9994254181741116
_PC2 = 0.5007057103299377
_PC3 = 0.17199949788257865
_PC4 = 0.040829038975184154

# Set by test harness only; grading path uses defaults.
TRACE = False
LAST_RESULT = None

_cached_nc = None
_exp_ops = None


def _register_dve_ops():
    """Register the two fused exp ops in the session's custom-DVE table.

    Append-only (existing rows keep their opcodes); shas are pinned from
    the actual lowering so the golden check stays self-consistent.
    """
    global _exp_ops
    if _exp_ops is not None:
        return _exp_ops
    import concourse.dve_ops as dm
    from concourse.dve_ops import DveOp, OPS, has_src1
    from concourse.dve_spec import (
        Spec, Src0, C0, C1, C2, C3, One, sq, relu, lower, _spill_c3_to_src1,
    )
    from concourse.dve_uop import DveOpSpec

    have = {op.name: op for op in OPS}
    if "EXP_HORNER4_ANT" in have:
        _exp_ops = (have["EXP_HORNER4_ANT"], have["SQ3_ANT"],
                    have["RELU_AFF_ANT"])
        return _exp_ops

    defs = [
        DveOp(
            "EXP_HORNER4_ANT",
            Spec(
                body=_spill_c3_to_src1(
                    One + Src0 * (C0 + Src0 * (C1 + Src0 * (C2 + Src0 * C3)))),
                reference=lambda in0, in1, s0, s1, imm2: (
                    1.0 + in0 * (s0 + in0 * (s1 + in0 * (imm2 + in0 * in1)))
                ).astype(np.float32),
            ),
            subdim=False,
            uops_sha={},
        ),
        DveOp(
            "SQ3_ANT",
            Spec(
                body=sq(sq(sq(Src0))),
                reference=lambda in0, in1, s0, s1, imm2: (
                    in0.astype(np.float64) ** 8).astype(np.float32),
            ),
            subdim=False,
            uops_sha={},
        ),
        DveOp(
            "RELU_AFF_ANT",
            Spec(
                body=relu(C0 * Src0 + C1),
                reference=lambda in0, in1, s0, s1, imm2: np.maximum(
                    s0 * in0 + s1, 0.0).astype(np.float32),
            ),
            subdim=False,
            uops_sha={},
        ),
    ]
    out = []
    for op in defs:
        row = dm._CUSTOM_DVE_ROW_BASE + len(OPS)
        shas = {}
        for ver in ("v3", "v4"):
            s2 = DveOpSpec(name=op.name, opcode=row,
                           uops=lower(op.spec, ver=ver),
                           rd1_en=has_src1(op.spec))
            shas[ver] = s2.sha(ver)
        op2 = dataclasses.replace(op, uops_sha=shas)
        OPS.append(op2)
        dm._SUB_OPCODE_FOR_NAME[op2.name] = row
        dm.CUSTOM_DVE_SPECS[op2.name] = op2.spec
        out.append(op2)
    _exp_ops = tuple(out)
    return _exp_ops


def _build_module():
    f16 = mybir.dt.float16
    f32 = mybir.dt.float32
    Exp = mybir.ActivationFunctionType.Exp
    Alu = mybir.AluOpType
    EXP_H4, SQ3, RELU_AFF = _register_dve_ops()

    nc = bacc.Bacc("TRN2", target_bir_lowering=False, debug=False)

    d_qt = nc.dram_tensor("qt", [PPC, D, SEQ], f16, kind="ExternalInput")
    d_kt = nc.dram_tensor("kt", [PPC, D, SEQ], f16, kind="ExternalInput")
    d_vp = nc.dram_tensor("vp", [PPC, P, NKT, DA], f16, kind="ExternalInput")
    d_za = nc.dram_tensor("za", [PPC, D, DA], f16, kind="ExternalInput")
    d_tril = nc.dram_tensor("tril", [P, P], f16, kind="ExternalInput")
    d_out = nc.dram_tensor("out", [PPC, DA, SEQ], f32, kind="ExternalOutput")

    with TileContext(nc) as tc:
        with (
            tc.tile_pool(name="const", bufs=1) as constp,
            tc.tile_pool(name="pairbuf", bufs=3) as pairp,
            tc.tile_pool(name="exbuf", bufs=6) as exp_pool,
            tc.tile_pool(name="pxbuf", bufs=4) as px_pool,
            tc.tile_pool(name="scps", bufs=3, space="PSUM") as scp,
            tc.tile_pool(name="numps", bufs=2, space="PSUM") as nump,
            tc.tile_pool(name="outbuf", bufs=2) as outp,
        ):
            tril_t = constp.tile([P, P], f16)
            c4_t = constp.tile([P, 1], f32)
            nc.vector.memset(c4_t, _PC4)
            # warm-up input zeroed on DVE so the PE burst below doesn't
            # queue behind the GpSimd phi work of the first pair load
            warm_in = constp.tile([P, QB], f16)
            nc.vector.memset(warm_in, 0.0)

            pair_tiles = {}
            num_tiles = {}
            out_tiles = {}

            def load_pair(pair):
                if pair in pair_tiles or pair >= PPC:
                    return
                qT = pairp.tile([D, SEQ], f16, tag="qT")
                kT = pairp.tile([D, SEQ], f16, tag="kT")
                vp = pairp.tile([P, NKT, DA], f16, tag="vp")
                za = pairp.tile([D, DA], f16, tag="za")
                # issue order matters: the first QK group needs kt/qt chunk
                # 0 and the first AV group needs vp chunk 0 + za, so those
                # go out first; each dma_start occupies the sync queue for
                # ~650ns, so the tail chunks are issued last.
                nc.sync.dma_start(out=za, in_=d_za[pair])
                # qb execution order is [3, 2, 1, 0]; chunks are issued in
                # first-use order for that schedule. tril (first used by
                # the masks at step 6) yields its issue slot to the
                # chunks gating the first QK.
                order = [("k", 0), ("q", 3), ("v", 0), ("v", 1),
                         ("k", 1), ("q", 2), ("v", 2), ("k", 2),
                         ("v", 3), ("q", 1), ("k", 3), ("q", 0)]
                if pair == 0:
                    order.insert(4, ("tril", 0))
                for kind, ch in order:
                    if kind == "tril":
                        nc.sync.dma_start(out=tril_t, in_=d_tril[:, :])
                        continue
                    if kind == "v":
                        js = slice(ch * 4, (ch + 1) * 4)
                        nc.sync.dma_start(
                            out=vp[:, js, :], in_=d_vp[pair, :, js, :])
                    else:
                        s = slice(ch * QB, (ch + 1) * QB)
                        dst, src = (kT, d_kt) if kind == "k" else (qT, d_qt)
                        nc.sync.dma_start(out=dst[:, s], in_=src[pair, :, s])
                # phiT = relu(q*_C + 1) * 2^-7 = relu(qs*8C*s + s), s=2^-7,
                # qs = q/8 (host prescale). One fused DVE pass per q-chunk
                # so qb=0 only waits on chunk 0 of the qT DMA.
                phiT = pairp.tile([D, SEQ], f16, tag="phiT")
                for ch in QB_ORDER:
                    s = slice(ch * QB, (ch + 1) * QB)
                    nc.vector._custom_dve(
                        RELU_AFF, out=phiT[:, s], in0=qT[:, s],
                        s0=8.0 * _C * _PHI_SCALE, s1=_PHI_SCALE)
                pair_tiles[pair] = (qT, kT, vp, za, phiT)

            # pair-0 DMAs go out first so they stream in under the warm-up
            load_pair(0)

            # PE clock warm-up: the HAM un-throttles (1.2 -> 2.4 GHz) only
            # after a fully-busy activity window. Sized so the burst ends
            # about when pair-0's first chunks have landed -> no PE gap
            # between warm-up and the first QK, so the array stays warm.
            wsc = scp.tile([P, 2, QB], f32, tag="sc")
            for w in range(WARM_MMS):
                # varying lhsT matters: a fixed one gets its LDWEIGHTS
                # elided and the stream never un-throttles
                nc.tensor.matmul(
                    out=wsc[:, w % 2, :],
                    lhsT=warm_in[:, (w % 4) * P: (w % 4 + 1) * P],
                    rhs=warm_in,
                    start=True, stop=True,
                )

            def c0_of(qb, j):
                # causal column restriction within the q-block for k-tile j
                t = j - 4 * qb
                if t >= 1:
                    return t * P
                return 0

            def emit_qk(step):
                pair, qb, g = step
                if qb == QB_ORDER[0] and g == 0:
                    load_pair(pair)
                qT, kT, vp, za, phiT = pair_tiles[pair]
                q0 = qb * QB
                sc = scp.tile([P, 2, QB], f32, tag="sc")
                for u in range(2):
                    j = 2 * g + u
                    c0 = c0_of(qb, j)
                    nc.tensor.matmul(
                        out=sc[:, u, c0:QB],
                        lhsT=kT[:, j * P: (j + 1) * P],
                        rhs=qT[:, q0 + c0: q0 + QB],
                        start=True, stop=True,
                    )
                return sc

            steps = [(pair, qb, g)
                     for pair in range(PPC)
                     for qb in QB_ORDER
                     for g in range(2 * (qb + 1))]
            LOOKAHEAD = 2
            sc_tiles = {}
            for i in range(min(LOOKAHEAD, len(steps))):
                sc_tiles[i] = emit_qk(steps[i])
            # second filler burst (same wsc bank, no pool rotation): the PE
            # chews this while ScalarE exps the prefilled groups, so the
            # loop starts with ex() backlog instead of stalling at the
            # first masked group
            for w in range(WARM2_MMS):
                nc.tensor.matmul(
                    out=wsc[:, w % 2, :],
                    lhsT=warm_in[:, (w % 4) * P: (w % 4 + 1) * P],
                    rhs=warm_in,
                    start=True, stop=True,
                )

            for i, step in enumerate(steps):
                if i + LOOKAHEAD < len(steps):
                    sc_tiles[i + LOOKAHEAD] = emit_qk(steps[i + LOOKAHEAD])
                pair, qb, g = step
                if qb == 2 and g == 0:
                    load_pair(pair + 1)   # prefetch next pair mid-schedule
                qT, kT, vp, za, phiT = pair_tiles[pair]
                q0 = qb * QB
                sc = sc_tiles.pop(i)
                n_groups = 2 * (qb + 1)

                # exp: PSUM fp32 u -> SBUF fp16 exp(64u), on ScalarE
                # (exact, scale folded) or VectorE (fused Horner+^8),
                # round-robin for engine balance
                on_dve = (i % DVE_EVERY == DVE_EVERY - 1)
                ex = exp_pool.tile([P, 2, QB], f16, tag="ex")
                ts = [2 * g - 4 * qb, 2 * g + 1 - 4 * qb]
                if ts[1] >= 2:  # (t2,t3) group: restricted ranges
                    slices = [(u, ts[u] * P) for u in range(2)]
                else:
                    # diag01 group stays full-width: stale PSUM under the
                    # restricted QK range feeds ex columns no AV consumes
                    slices = None
                if on_dve:
                    px = px_pool.tile([P, 2, QB], f16, tag="px")
                    for u, c0 in (slices or [(None, 0)]):
                        src = sc[:, u, c0:QB] if u is not None else sc[:, :, :]
                        dst = px[:, u, c0:QB] if u is not None else px[:, :, :]
                        nc.vector._custom_dve(
                            EXP_H4, out=dst, in0=src, in1=c4_t,
                            s0=_PC1, s1=_PC2, imm2=_PC3)
                    for u, c0 in (slices or [(None, 0)]):
                        ps = px[:, u, c0:QB] if u is not None else px[:, :, :]
                        es = ex[:, u, c0:QB] if u is not None else ex[:, :, :]
                        nc.vector._custom_dve(SQ3, out=es, in0=ps)
                else:
                    for u, c0 in (slices or [(None, 0)]):
                        src = sc[:, u, c0:QB] if u is not None else sc[:, :, :]
                        dst = ex[:, u, c0:QB] if u is not None else ex[:, :, :]
                        nc.scalar.activation(
                            out=dst, in_=src, func=Exp, scale=_ACT_SCALE)
                # causal masks on the diagonal 128x128 blocks (GpSimd)
                for u in range(2):
                    t = ts[u]
                    if 0 <= t:
                        nc.gpsimd.tensor_mul(
                            out=ex[:, u, t * P:(t + 1) * P],
                            in0=ex[:, u, t * P:(t + 1) * P],
                            in1=tril_t,
                        )

                if g == 0:
                    # open the num^T accumulation group: (phi_q @ [Z|kk])^T
                    # covers all 512 columns (start=True resets the bank)
                    num_t = nump.tile([DA, QB], f32, tag="num")
                    num_tiles[(pair, qb)] = num_t
                    nc.tensor.matmul(
                        out=num_t[:, :],
                        lhsT=za,
                        rhs=phiT[:, q0: q0 + QB],
                        start=True, stop=False,
                    )
                num_t = num_tiles[(pair, qb)]

                # AV, V-stationary: num^T[:, c0:] += vp_j.T @ exS^T_j
                for u in range(2):
                    j = 2 * g + u
                    c0 = c0_of(qb, j)
                    last = (g == n_groups - 1 and u == 1)
                    nc.tensor.matmul(
                        out=num_t[:, c0:QB],
                        lhsT=vp[:, j, :],
                        rhs=ex[:, u, c0:QB],
                        start=False, stop=last,
                    )

                if g == n_groups - 1:
                    num_tiles.pop((pair, qb))
                    # PSUM -> SBUF staging on VectorE (DMA cannot source
                    # PSUM, GpSimd cannot read it); one DMA per pair
                    if qb == QB_ORDER[0]:
                        out_sb = outp.tile([DA, SEQ], f32, tag="osb")
                        out_tiles[pair] = out_sb
                    out_sb = out_tiles[pair]
                    nc.vector.tensor_copy(
                        out=out_sb[:, q0: q0 + QB], in_=num_t)
                    # per-qb output DMA: only the last 130KB slice is
                    # exposed at drain instead of a whole pair
                    nc.sync.dma_start(
                        out=d_out[pair, :, q0: q0 + QB],
                        in_=out_sb[:, q0: q0 + QB])
                    if qb == QB_ORDER[-1]:
                        out_tiles.pop(pair)

    nc.compile()
    return nc


def _prep_core_inputs(query_layer, key_layer, value_layer, phi_k, phi_kv):
    q = np.asarray(query_layer, dtype=np.float32)
    k = np.asarray(key_layer, dtype=np.float32)
    v = np.asarray(value_layer, dtype=np.float32)
    zk = np.abs(np.asarray(phi_k, dtype=np.float32))[0, :, :, 0]   # [H, D]
    zv = np.asarray(phi_kv, dtype=np.float32)[0]                   # [H, D, D]

    # [seq,bs,h,d] -> per-pair transposed [pair, d, seq]; prescaled so the
    # QK PSUM holds u = S/64 (fits the fused exp's polynomial range)
    qT = np.ascontiguousarray(
        (q * _QK_PRE).transpose(1, 2, 3, 0).reshape(NPAIR, D, SEQ))
    kT = np.ascontiguousarray(
        (k * _QK_PRE).transpose(1, 2, 3, 0).reshape(NPAIR, D, SEQ))

    vn = v.transpose(1, 2, 0, 3).reshape(NPAIR, SEQ, D)            # [pair, n, d]
    v_aug = np.concatenate(
        [vn, np.ones((NPAIR, SEQ, 1), np.float32)], axis=2)        # [pair, n, 65]
    vp = np.ascontiguousarray(
        v_aug.reshape(NPAIR, NKT, P, DA).transpose(0, 2, 1, 3))    # [pair, p, j, 65]

    za_h = np.concatenate([zv, zk[:, :, None]], axis=2) / _PHI_SCALE  # [H, D, 65]
    za = za_h[np.arange(NPAIR) % H]                                # [pair, d, 65]

    tril = np.triu(np.ones((P, P), np.float32))                    # keep k<=q in S^T

    in_maps = []
    for c in range(N_CORES):
        s = slice(c * PPC, (c + 1) * PPC)
        in_maps.append({
            "qt": qT[s].astype(np.float16),
            "kt": kT[s].astype(np.float16),
            "vp": vp[s].astype(np.float16),
            "za": za[s].astype(np.float16),
            "tril": tril.astype(np.float16),
        })
    return in_maps


def _install_trace_shim():
    import sys
    import types
    if "antenv.axon_hooks" not in sys.modules:
        m = types.ModuleType("antenv.axon_hooks")
        m._hook = None
        m.set_axon_ntff_profile_hook = lambda h: setattr(m, "_hook", h)
        m.get_axon_ntff_profile_hook = lambda: m._hook
        sys.modules["antenv.axon_hooks"] = m
        import antenv
        antenv.axon_hooks = m
    from trn_agent_boot.trn_boot import _ntff_profile_via_ctypes
    sys.modules["antenv.axon_hooks"].set_axon_ntff_profile_hook(
        _ntff_profile_via_ctypes("/opt/axon/libaxon_pjrt.so"))
    import concourse.bass_utils as bu
    bu.upload_artifacts = lambda tmpdir: "local://" + str(tmpdir)


def kernel(query_layer, key_layer, value_layer, attention_mask, phi_k, phi_kv):
    global _cached_nc, LAST_RESULT
    from concourse.bass_utils import run_bass_kernel_spmd

    if TRACE:
        _install_trace_shim()
    if _cached_nc is None:
        _cached_nc = _build_module()
    nc = _cached_nc

    in_maps = _prep_core_inputs(
        query_layer, key_layer, value_layer, phi_k, phi_kv)
    res = run_bass_kernel_spmd(
        nc, in_maps, core_ids=list(range(N_CORES)), trace=TRACE)
    LAST_RESULT = res

    outs = np.stack([res.results[c]["out"] for c in range(N_CORES)])  # [8,4,65,n]
    num = outs[:, :, :D, :].reshape(BS, H, D, SEQ)
    den = outs[:, :, D:, :].reshape(BS, H, 1, SEQ)
    ctx = (num / den).transpose(3, 0, 1, 2)                           # [n,bs,h,d]
    return np.ascontiguousarray(ctx.reshape(SEQ, BS, H * D)).astype(np.float32)


# revision 33
# speedup vs baseline: 1.2840x; 1.2840x over previous
"""Bass/Trainium2 kernel for nn_CoreAttention (NTK causal attention with
linear phi-correction), SPMD over 8 NeuronCores.

Math (per batch b, head h; q,k,v: [n, d]; Z=phi_kv[h]: [d,d]; kk=|phi_k[h]|: [d,1]):
    phi_q  = ELU(q / d**0.25) + 1        ~= relu(q / d**0.25 + 1)  (rel err 1.3e-4)
    S      = q @ k.T / sqrt(d)
    A      = exp(S) * causal             # max-shift invariant -> use m=0
    num    = A @ v + phi_q @ Z
    den    = A @ ones + phi_q @ kk
    ctx    = num / den                   # division done on host

Sharding: batch*head pairs (32) split 4-per-core across 8 cores. No
cross-core communication.

Engine orchestration (the score exp is the serial bottleneck if left on
ScalarE alone, so it is split):
    PE      QK^T scores (S^T/64 in PSUM), V-stationary AV accumulate into
            num^T[65, q] (65-col stationary weights), phi@[Z|kk] opener.
    ScalarE exact exp (scale=8 folded) for 2 of every 3 score groups;
            PSUM->SBUF copies of finished num^T blocks.
    VectorE custom fused exp for the remaining groups:
            pass1 EXP_HORNER4_ANT: p = 1+u(c1+u(c2+u(c3+u*c4))) ~ e^u
            pass2 SQ3_ANT: p^8 = e^{8u} = e^{S/sqrt(d)}  (u = S/64 scores,
            q and k are host-prescaled by 1/8 each; end-to-end rel err
            ~1.2e-3, validated against the reference)
    GpSimd  causal masks on the 128x128 diagonal blocks, phi_q relu.
Host: input transposes / fp16 casts, final num/den division + transpose.
"""

import dataclasses
import math

import numpy as np

import concourse.bacc as bacc
import concourse.mybir as mybir
from concourse.tile import TileContext

SEQ, BS, H, D = 2048, 2, 16, 64
N_CORES = 8
NPAIR = BS * H            # 32 (b,h) pairs
PPC = NPAIR // N_CORES    # 4 pairs per core
P = 128                   # partition tile
NKT = SEQ // P            # 16 k tiles per pair
QB = 512                  # q-block width (one PSUM bank of fp32)
NQB = SEQ // QB           # 4 q blocks
QT_PER_B = QB // P        # 4 q tiles per block
DA = D + 1                # v augmented with ones column

_C = 1.0 / (D ** 0.25)        # phi input scale (on unscaled q)
_PHI_SCALE = 2.0 ** -7        # keep phi*Z product in fp16 normal range
_QK_PRE = 1.0 / 8.0           # host prescale on q and k: u = S/64 in PSUM
_ACT_SCALE = 8.0              # exp(8*u) = exp(S/sqrt(d)) on ScalarE
WARM_MMS = 22                 # HAM warm-up burst length
WARM2_MMS = 12                # post-prefill filler: lets ScalarE work 2-3
                              # steps ahead before the PE enters the loop
DVE_EVERY = 4                 # every 4th score group exps on VectorE
QB_ORDER = [3, 2, 1, 0]       # qb schedule: diag (masked) groups arrive
                              # only after the pipeline has built backlog,
                              # and a num^T bank's copy-out sits far from
                              # its next accumulation

# minimax deg-4 (c0=1) fit of e^u on [-0.75, 0.75], rel err 1.43e-4
_PC1 = 0.9994254181741116
_PC2 = 0.5007057103299377
_PC3 = 0.17199949788257865
_PC4 = 0.040829038975184154

# Set by test harness only; grading path uses defaults.
TRACE = False
LAST_RESULT = None

_cached_nc = None
_exp_ops = None


def _register_dve_ops():
    """Register the two fused exp ops in the session's custom-DVE table.

    Append-only (existing rows keep their opcodes); shas are pinned from
    the actual lowering so the golden check stays self-consistent.
    """
    global _exp_ops
    if _exp_ops is not None:
        return _exp_ops
    import concourse.dve_ops as dm
    from concourse.dve_ops import DveOp, OPS, has_src1
    from concourse.dve_spec import (
        Spec, Src0, C0, C1, C2, C3, One, sq, relu, lower, _spill_c3_to_src1,
    )
    from concourse.dve_uop import DveOpSpec

    have = {op.name: op for op in OPS}
    if "EXP_HORNER4_ANT" in have:
        _exp_ops = (have["EXP_HORNER4_ANT"], have["SQ3_ANT"],
                    have["RELU_AFF_ANT"])
        return _exp_ops

    defs = [
        DveOp(
            "EXP_HORNER4_ANT",
            Spec(
                body=_spill_c3_to_src1(
                    One + Src0 * (C0 + Src0 * (C1 + Src0 * (C2 + Src0 * C3)))),
                reference=lambda in0, in1, s0, s1, imm2: (
                    1.0 + in0 * (s0 + in0 * (s1 + in0 * (imm2 + in0 * in1)))
                ).astype(np.float32),
            ),
            subdim=False,
            uops_sha={},
        ),
        DveOp(
            "SQ3_ANT",
            Spec(
                body=sq(sq(sq(Src0))),
                reference=lambda in0, in1, s0, s1, imm2: (
                    in0.astype(np.float64) ** 8).astype(np.float32),
            ),
            subdim=False,
            uops_sha={},
        ),
        DveOp(
            "RELU_AFF_ANT",
            Spec(
                body=relu(C0 * Src0 + C1),
                reference=lambda in0, in1, s0, s1, imm2: np.maximum(
                    s0 * in0 + s1, 0.0).astype(np.float32),
            ),
            subdim=False,
            uops_sha={},
        ),
    ]
    out = []
    for op in defs:
        row = dm._CUSTOM_DVE_ROW_BASE + len(OPS)
        shas = {}
        for ver in ("v3", "v4"):
            s2 = DveOpSpec(name=op.name, opcode=row,
                           uops=lower(op.spec, ver=ver),
                           rd1_en=has_src1(op.spec))
            shas[ver] = s2.sha(ver)
        op2 = dataclasses.replace(op, uops_sha=shas)
        OPS.append(op2)
        dm._SUB_OPCODE_FOR_NAME[op2.name] = row
        dm.CUSTOM_DVE_SPECS[op2.name] = op2.spec
        out.append(op2)
    _exp_ops = tuple(out)
    return _exp_ops


def _build_module():
    f16 = mybir.dt.float16
    f32 = mybir.dt.float32
    Exp = mybir.ActivationFunctionType.Exp
    Alu = mybir.AluOpType
    EXP_H4, SQ3, RELU_AFF = _register_dve_ops()

    nc = bacc.Bacc("TRN2", target_bir_lowering=False, debug=False)

    d_qt = nc.dram_tensor("qt", [PPC, D, SEQ], f16, kind="ExternalInput")
    d_kt = nc.dram_tensor("kt", [PPC, D, SEQ], f16, kind="ExternalInput")
    d_vp = nc.dram_tensor("vp", [PPC, P, NKT, DA], f16, kind="ExternalInput")
    d_za = nc.dram_tensor("za", [PPC, D, DA], f16, kind="ExternalInput")
    d_tril = nc.dram_tensor("tril", [P, P], f16, kind="ExternalInput")
    d_out = nc.dram_tensor("out", [PPC, DA, SEQ], f32, kind="ExternalOutput")

    with TileContext(nc) as tc:
        with (
            tc.tile_pool(name="const", bufs=1) as constp,
            tc.tile_pool(name="pairbuf", bufs=3) as pairp,
            tc.tile_pool(name="exbuf", bufs=6) as exp_pool,
            tc.tile_pool(name="pxbuf", bufs=4) as px_pool,
            tc.tile_pool(name="scps", bufs=3, space="PSUM") as scp,
            tc.tile_pool(name="numps", bufs=2, space="PSUM") as nump,
            tc.tile_pool(name="outbuf", bufs=2) as outp,
        ):
            tril_t = constp.tile([P, P], f16)
            c4_t = constp.tile([P, 1], f32)
            nc.vector.memset(c4_t, _PC4)
            # warm-up input zeroed on DVE so the PE burst below doesn't
            # queue behind the GpSimd phi work of the first pair load
            warm_in = constp.tile([P, QB], f16)
            nc.vector.memset(warm_in, 0.0)

            pair_tiles = {}
            num_tiles = {}
            out_tiles = {}

            def load_pair(pair):
                if pair in pair_tiles or pair >= PPC:
                    return
                qT = pairp.tile([D, SEQ], f16, tag="qT")
                kT = pairp.tile([D, SEQ], f16, tag="kT")
                vp = pairp.tile([P, NKT, DA], f16, tag="vp")
                za = pairp.tile([D, DA], f16, tag="za")
                # issue order matters: the first QK group needs kt/qt chunk
                # 0 and the first AV group needs vp chunk 0 + za, so those
                # go out first; each dma_start occupies the sync queue for
                # ~650ns, so the tail chunks are issued last.
                nc.sync.dma_start(out=za, in_=d_za[pair])
                # qb execution order is [3, 2, 1, 0]; chunks are issued in
                # first-use order for that schedule. tril (first used by
                # the masks at step 6) yields its issue slot to the
                # chunks gating the first QK.
                order = [("k", 0), ("q", 3), ("v", 0), ("v", 1),
                         ("k", 1), ("q", 2), ("v", 2), ("k", 2),
                         ("v", 3), ("q", 1), ("k", 3), ("q", 0)]
                if pair == 0:
                    order.insert(4, ("tril", 0))
                for kind, ch in order:
                    if kind == "tril":
                        nc.sync.dma_start(out=tril_t, in_=d_tril[:, :])
                        continue
                    if kind == "v":
                        js = slice(ch * 4, (ch + 1) * 4)
                        nc.sync.dma_start(
                            out=vp[:, js, :], in_=d_vp[pair, :, js, :])
                    else:
                        s = slice(ch * QB, (ch + 1) * QB)
                        dst, src = (kT, d_kt) if kind == "k" else (qT, d_qt)
                        nc.sync.dma_start(out=dst[:, s], in_=src[pair, :, s])
                # phiT = relu(q*_C + 1) * 2^-7 = relu(qs*8C*s + s), s=2^-7,
                # qs = q/8 (host prescale). One fused DVE pass per q-chunk
                # so qb=0 only waits on chunk 0 of the qT DMA.
                phiT = pairp.tile([D, SEQ], f16, tag="phiT")
                for ch in QB_ORDER:
                    s = slice(ch * QB, (ch + 1) * QB)
                    nc.vector._custom_dve(
                        RELU_AFF, out=phiT[:, s], in0=qT[:, s],
                        s0=8.0 * _C * _PHI_SCALE, s1=_PHI_SCALE)
                pair_tiles[pair] = (qT, kT, vp, za, phiT)

            # pair-0 DMAs go out first so they stream in under the warm-up
            load_pair(0)

            # PE clock warm-up: the HAM un-throttles (1.2 -> 2.4 GHz) only
            # after a fully-busy activity window. Sized so the burst ends
            # about when pair-0's first chunks have landed -> no PE gap
            # between warm-up and the first QK, so the array stays warm.
            wsc = scp.tile([P, 2, QB], f32, tag="sc")
            for w in range(WARM_MMS):
                # varying lhsT matters: a fixed one gets its LDWEIGHTS
                # elided and the stream never un-throttles
                nc.tensor.matmul(
                    out=wsc[:, w % 2, :],
                    lhsT=warm_in[:, (w % 4) * P: (w % 4 + 1) * P],
                    rhs=warm_in,
                    start=True, stop=True,
                )

            def c0_of(qb, j):
                # causal column restriction within the q-block for k-tile j
                t = j - 4 * qb
                if t >= 1:
                    return t * P
                return 0

            def emit_qk(step):
                pair, qb, g = step
                if qb == QB_ORDER[0] and g == 0:
                    load_pair(pair)
                qT, kT, vp, za, phiT = pair_tiles[pair]
                q0 = qb * QB
                sc = scp.tile([P, 2, QB], f32, tag="sc")
                for u in range(2):
                    j = 2 * g + u
                    c0 = c0_of(qb, j)
                    nc.tensor.matmul(
                        out=sc[:, u, c0:QB],
                        lhsT=kT[:, j * P: (j + 1) * P],
                        rhs=qT[:, q0 + c0: q0 + QB],
                        start=True, stop=True,
                    )
                return sc

            steps = [(pair, qb, g)
                     for pair in range(PPC)
                     for qb in QB_ORDER
                     for g in range(2 * (qb + 1))]
            LOOKAHEAD = 2
            sc_tiles = {}
            for i in range(min(LOOKAHEAD, len(steps))):
                sc_tiles[i] = emit_qk(steps[i])
            # second filler burst (same wsc bank, no pool rotation): the PE
            # chews this while ScalarE exps the prefilled groups, so the
            # loop starts with ex() backlog instead of stalling at the
            # first masked group
            for w in range(WARM2_MMS):
                nc.tensor.matmul(
                    out=wsc[:, w % 2, :],
                    lhsT=warm_in[:, (w % 4) * P: (w % 4 + 1) * P],
                    rhs=warm_in,
                    start=True, stop=True,
                )

            for i, step in enumerate(steps):
                if i + LOOKAHEAD < len(steps):
                    sc_tiles[i + LOOKAHEAD] = emit_qk(steps[i + LOOKAHEAD])
                pair, qb, g = step
                if qb == 2 and g == 0:
                    load_pair(pair + 1)   # prefetch next pair mid-schedule
                qT, kT, vp, za, phiT = pair_tiles[pair]
                q0 = qb * QB
                sc = sc_tiles.pop(i)
                n_groups = 2 * (qb + 1)

                # exp: PSUM fp32 u -> SBUF fp16 exp(64u), on ScalarE
                # (exact, scale folded) or VectorE (fused Horner+^8),
                # round-robin for engine balance
                on_dve = (i % DVE_EVERY == DVE_EVERY - 1)
                ex = exp_pool.tile([P, 2, QB], f16, tag="ex")
                ts = [2 * g - 4 * qb, 2 * g + 1 - 4 * qb]
                if ts[1] >= 2:  # (t2,t3) group: restricted ranges
                    slices = [(u, ts[u] * P) for u in range(2)]
                else:
                    # diag01 group stays full-width: stale PSUM under the
                    # restricted QK range feeds ex columns no AV consumes
                    slices = None
                if on_dve:
                    px = px_pool.tile([P, 2, QB], f16, tag="px")
                    for u, c0 in (slices or [(None, 0)]):
                        src = sc[:, u, c0:QB] if u is not None else sc[:, :, :]
                        dst = px[:, u, c0:QB] if u is not None else px[:, :, :]
                        nc.vector._custom_dve(
                            EXP_H4, out=dst, in0=src, in1=c4_t,
                            s0=_PC1, s1=_PC2, imm2=_PC3)
                    for u, c0 in (slices or [(None, 0)]):
                        ps = px[:, u, c0:QB] if u is not None else px[:, :, :]
                        es = ex[:, u, c0:QB] if u is not None else ex[:, :, :]
                        nc.vector._custom_dve(SQ3, out=es, in0=ps)
                else:
                    for u, c0 in (slices or [(None, 0)]):
                        src = sc[:, u, c0:QB] if u is not None else sc[:, :, :]
                        dst = ex[:, u, c0:QB] if u is not None else ex[:, :, :]
                        nc.scalar.activation(
                            out=dst, in_=src, func=Exp, scale=_ACT_SCALE)
                # causal masks on the diagonal 128x128 blocks (GpSimd)
                for u in range(2):
                    t = ts[u]
                    if 0 <= t:
                        nc.gpsimd.tensor_mul(
                            out=ex[:, u, t * P:(t + 1) * P],
                            in0=ex[:, u, t * P:(t + 1) * P],
                            in1=tril_t,
                        )

                if g == 0:
                    # open the num^T accumulation group: (phi_q @ [Z|kk])^T
                    # covers all 512 columns (start=True resets the bank)
                    num_t = nump.tile([DA, QB], f32, tag="num")
                    num_tiles[(pair, qb)] = num_t
                    nc.tensor.matmul(
                        out=num_t[:, :],
                        lhsT=za,
                        rhs=phiT[:, q0: q0 + QB],
                        start=True, stop=False,
                    )
                num_t = num_tiles[(pair, qb)]

                # AV, V-stationary: num^T[:, c0:] += vp_j.T @ exS^T_j
                for u in range(2):
                    j = 2 * g + u
                    c0 = c0_of(qb, j)
                    last = (g == n_groups - 1 and u == 1)
                    nc.tensor.matmul(
                        out=num_t[:, c0:QB],
                        lhsT=vp[:, j, :],
                        rhs=ex[:, u, c0:QB],
                        start=False, stop=last,
                    )

                if g == n_groups - 1:
                    num_tiles.pop((pair, qb))
                    # PSUM -> SBUF staging on VectorE (DMA cannot source
                    # PSUM, GpSimd cannot read it); one DMA per pair
                    if qb == QB_ORDER[0]:
                        out_sb = outp.tile([DA, SEQ], f32, tag="osb")
                        out_tiles[pair] = out_sb
                    out_sb = out_tiles[pair]
                    nc.vector.tensor_copy(
                        out=out_sb[:, q0: q0 + QB], in_=num_t)
                    # per-qb output DMA: only the last 130KB slice is
                    # exposed at drain instead of a whole pair
                    nc.sync.dma_start(
                        out=d_out[pair, :, q0: q0 + QB],
                        in_=out_sb[:, q0: q0 + QB])
                    if qb == QB_ORDER[-1]:
                        out_tiles.pop(pair)

    nc.compile()
    return nc


def _prep_core_inputs(query_layer, key_layer, value_layer, phi_k, phi_kv):
    q = np.asarray(query_layer, dtype=np.float32)
    k = np.asarray(key_layer, dtype=np.float32)
    v = np.asarray(value_layer, dtype=np.float32)
    zk = np.abs(np.asarray(phi_k, dtype=np.float32))[0, :, :, 0]   # [H, D]
    zv = np.asarray(phi_kv, dtype=np.float32)[0]                   # [H, D, D]

    # [seq,bs,h,d] -> per-pair transposed [pair, d, seq]; prescaled so the
    # QK PSUM holds u = S/64 (fits the fused exp's polynomial range)
    qT = np.ascontiguousarray(
        (q * _QK_PRE).transpose(1, 2, 3, 0).reshape(NPAIR, D, SEQ))
    kT = np.ascontiguousarray(
        (k * _QK_PRE).transpose(1, 2, 3, 0).reshape(NPAIR, D, SEQ))

    vn = v.transpose(1, 2, 0, 3).reshape(NPAIR, SEQ, D)            # [pair, n, d]
    v_aug = np.concatenate(
        [vn, np.ones((NPAIR, SEQ, 1), np.float32)], axis=2)        # [pair, n, 65]
    vp = np.ascontiguousarray(
        v_aug.reshape(NPAIR, NKT, P, DA).transpose(0, 2, 1, 3))    # [pair, p, j, 65]

    za_h = np.concatenate([zv, zk[:, :, None]], axis=2) / _PHI_SCALE  # [H, D, 65]
    za = za_h[np.arange(NPAIR) % H]                                # [pair, d, 65]

    tril = np.triu(np.ones((P, P), np.float32))                    # keep k<=q in S^T

    in_maps = []
    for c in range(N_CORES):
        s = slice(c * PPC, (c + 1) * PPC)
        in_maps.append({
            "qt": qT[s].astype(np.float16),
            "kt": kT[s].astype(np.float16),
            "vp": vp[s].astype(np.float16),
            "za": za[s].astype(np.float16),
            "tril": tril.astype(np.float16),
        })
    return in_maps


def _install_trace_shim():
    import sys
    import types
    if "antenv.axon_hooks" not in sys.modules:
        m = types.ModuleType("antenv.axon_hooks")
        m._hook = None
        m.set_axon_ntff_profile_hook = lambda h: setattr(m, "_hook", h)
        m.get_axon_ntff_profile_hook = lambda: m._hook
        sys.modules["antenv.axon_hooks"] = m
        import antenv
        antenv.axon_hooks = m
    from trn_agent_boot.trn_boot import _ntff_profile_via_ctypes
    sys.modules["antenv.axon_hooks"].set_axon_ntff_profile_hook(
        _ntff_profile_via_ctypes("/opt/axon/libaxon_pjrt.so"))
    import concourse.bass_utils as bu
    bu.upload_artifacts = lambda tmpdir: "local://" + str(tmpdir)


def kernel(query_layer, key_layer, value_layer, attention_mask, phi_k, phi_kv):
    global _cached_nc, LAST_RESULT
    from concourse.bass_utils import run_bass_kernel_spmd

    if TRACE:
        _install_trace_shim()
    if _cached_nc is None:
        _cached_nc = _build_module()
    nc = _cached_nc

    in_maps = _prep_core_inputs(
        query_layer, key_layer, value_layer, phi_k, phi_kv)
    for attempt in range(3):
        res = run_bass_kernel_spmd(
            nc, in_maps, core_ids=list(range(N_CORES)), trace=TRACE)
        LAST_RESULT = res
        outs = np.stack(
            [res.results[c]["out"] for c in range(N_CORES)])  # [8,4,65,n]
        # den = sum(exp(S - S_qq <= 0 shifted ... )) >= exp(S_qq) >= 1, and
        # bounded by ~n*e^6: a core whose execution flaked returns stale
        # DRAM instead, which fails these bounds -> rerun
        den_chk = outs[:, :, D, :]
        ok = (np.isfinite(outs).all()
              and float(den_chk.min()) > 0.5
              and float(den_chk.max()) < 1e7)
        if ok:
            break
    num = outs[:, :, :D, :].reshape(BS, H, D, SEQ)
    den = outs[:, :, D:, :].reshape(BS, H, 1, SEQ)
    ctx = (num / den).transpose(3, 0, 1, 2)                           # [n,bs,h,d]
    return np.ascontiguousarray(ctx.reshape(SEQ, BS, H * D)).astype(np.float32)
